# revision 23
# baseline (speedup 1.0000x reference)
"""Trainium2 Bass kernel for nn_BeliefPropagationCV (belief-propagation edge update).

Computes  y = 0.5 * ((mask * input_weight) @ input + llr_expander @ (llr_weight * llr))
for E = 4096 edges on 8 NeuronCores.

Sharding: row-shard the three [E, E] matrices (split output dim E into 8 slices of
512 rows); replicate the small vectors. Each core's shard is fed TRANSPOSED
(contraction dim j on SBUF partitions) so the TensorEngine performs the
x-weighted reduction directly via PSUM accumulation:

    y[i] = sum_j (mask.T*W.T)[j,i] * x[j] + sum_j E.T[j,i] * v[j],  v = llr_w*llr

Per 128-row j-chunk k: matmul(psum[1,512], lhsT=x[:,k:k+1], rhs=P_tile) accumulates.
The only elementwise work is one fp16 DVE multiply (mask ⊙ W) per tile.

mask / llr_expander are 0/1-valued, so the host-side fp16 cast is exact; W/x/v
are rounded to fp16 (~2^-11 relative), accumulation is fp32 in PSUM.
"""

import numpy as np

E = 4096
N_CORES = 8
R = E // N_CORES      # 512 output rows per core
P = 128               # SBUF partitions
K = E // P            # 32 contraction chunks of 128
OUTER = 8             # outer tiles (DMA/compute pipeline stages)
CPO = K // OUTER      # 4 chunks per outer tile
FREE = CPO * R        # 2048 fp16 elements per partition per outer tile


def _build_program():
    import concourse.bass as bass
    import concourse.tile as tile
    from concourse import bacc, mybir
    from contextlib import ExitStack

    f8 = mybir.dt.float8e4
    f16 = mybir.dt.float16
    f32 = mybir.dt.float32

    nc = bacc.Bacc(None)
    wt = nc.dram_tensor("wt", [OUTER, P, FREE], f16, kind="ExternalInput")
    # mask / llr_expander are 0/1-valued: fp8_e4m3 is exact and halves traffic.
    mt = nc.dram_tensor("mt", [OUTER, P, FREE], f8, kind="ExternalInput")
    et = nc.dram_tensor("et", [OUTER, P, FREE], f8, kind="ExternalInput")
    xcm = nc.dram_tensor("xcm", [P, K], f16, kind="ExternalInput")
    lvw = nc.dram_tensor("lvw", [P, 2 * K], f32, kind="ExternalInput")
    y = nc.dram_tensor("y", [R], f32, kind="ExternalOutput")

    with ExitStack() as ctx:
        tc = ctx.enter_context(tile.TileContext(nc))
        # bufs = OUTER: every shard tile resident at once (about 10.5 MB of
        # SBUF) so the DMA stream never stalls on slot reuse.
        singles = ctx.enter_context(tc.tile_pool(name="singles", bufs=1))
        wp = ctx.enter_context(tc.tile_pool(name="wp", bufs=OUTER))
        mp = ctx.enter_context(tc.tile_pool(name="mp", bufs=OUTER))
        ep = ctx.enter_context(tc.tile_pool(name="ep", bufs=OUTER))
        pp = ctx.enter_context(tc.tile_pool(name="pp", bufs=OUTER))
        psp = ctx.enter_context(tc.tile_pool(name="psp", bufs=1, space="PSUM"))

        # PE warm-up: the HAM clock gate keeps the PE at 1.2 GHz until it has
        # been busy ~3.4us. Run zero matmuls into a scratch PSUM bank during
        # the DMA ramp so the real matmuls run at 2.4 GHz.
        zmov = singles.tile([P, R], f16)
        nc.vector.memset(zmov, 0.0)
        zps = psp.tile([1, R], f32)
        for _ in range(28):
            nc.tensor.matmul(zps, zmov[:, :1], zmov, start=True, stop=True)

        # Small replicated vectors first on the ACT ring (tiny). Column-major
        # ([p, k] = elem k*128+p) so contraction chunk k is SBUF column k.
        xh = singles.tile([P, K], f16)
        nc.scalar.dma_start(out=xh, in_=xcm[:, :])
        lvf = singles.tile([P, 2 * K], f32)
        nc.scalar.dma_start(out=lvf, in_=lvw[:, :])
        vh = singles.tile([P, K], f16)
        nc.vector.tensor_mul(vh, lvf[:, :K], lvf[:, K:])

        # Per-tile interleaved loads: W on the SP ring; mask+expander on the
        # ACT ring right behind the small vectors.
        w_sbs, m_sbs, e_sbs = [], [], []
        for t in range(OUTER):
            w_sb = wp.tile([P, FREE], f16)
            nc.sync.dma_start(out=w_sb, in_=wt[t])
            m_sb = mp.tile([P, FREE], f8)
            nc.scalar.dma_start(out=m_sb, in_=mt[t])
            e_sb = ep.tile([P, FREE], f8)
            nc.scalar.dma_start(out=e_sb, in_=et[t])
            w_sbs.append(w_sb); m_sbs.append(m_sb); e_sbs.append(e_sb)

        ps = psp.tile([1, R], f32)
        n_mm = OUTER * CPO * 2
        i_mm = 0
        for t in range(OUTER):
            # Mixed-dtype multiply fp16 W x fp8 mask -> fp16. The DVE runs at
            # 1 elem/cycle/lane on mixed dtypes, so GpSimd takes the last
            # chunk in parallel while the DVE does the first three (split so
            # chunk-0 matmuls overlap the rest of the multiply).
            p_sb = pp.tile([P, FREE], f16)
            gsl = bass.ts(CPO - 1, R)
            nc.gpsimd.tensor_mul(p_sb[:, gsl], w_sbs[t][:, gsl], m_sbs[t][:, gsl])
            for lo, hi in ((0, 1), (1, CPO - 1)):
                hsl = bass.ds(lo * R, (hi - lo) * R)
                nc.vector.tensor_mul(p_sb[:, hsl], w_sbs[t][:, hsl], m_sbs[t][:, hsl])
            for c in range(CPO):
                k = t * CPO + c
                sl = bass.ts(c, R)
                nc.tensor.matmul(
                    ps, xh[:, k : k + 1], p_sb[:, sl],
                    start=(i_mm == 0), stop=(i_mm == n_mm - 1),
                )
                i_mm += 1
                nc.tensor.matmul(
                    ps, vh[:, k : k + 1], e_sbs[t][:, sl],
                    start=False, stop=(i_mm == n_mm - 1),
                )
                i_mm += 1

        # 0.5 * (term1 + term2) applied once on the tiny epilogue copy (DVE,
        # not ACT: using the scalar engine would pull in its activation-table
        # preamble load, delaying the ACT HWDGE ring's first data transfer).
        ysb = singles.tile([1, R], f32)
        nc.vector.tensor_scalar_mul(ysb, ps, 0.5)
        nc.sync.dma_start(out=y[:], in_=ysb)

    # bacc passes: splits multi-waits into event semaphores (TRN2 allows at
    # most one sync wait per instruction), register allocation, etc.
    nc.compile()
    return nc


def _prep_matrix(a_rows: np.ndarray, dtype=np.float16) -> np.ndarray:
    """[R, E] float -> [OUTER, P, FREE] with j on partitions.

    dram[t, p, c*R + i] = a_rows[i, (t*CPO + c)*P + p]
    """
    at = a_rows.astype(dtype).T  # [E, R]
    return np.ascontiguousarray(
        at.reshape(OUTER, CPO, P, R).transpose(0, 2, 1, 3)
    ).reshape(OUTER, P, FREE)


def _f8_dtype():
    from concourse import mybir

    return mybir.dt.np(mybir.dt.float8e4)


def _col_major_vec(v: np.ndarray, dtype=np.float32) -> np.ndarray:
    """[E] -> [P, K] with [p, k] = v[k*P + p]."""
    return np.ascontiguousarray(v.reshape(K, P).T.astype(dtype))


def _make_in_maps(input, input_weight, mask, llr, llr_weight, llr_expander):
    f8 = _f8_dtype()
    xcm = _col_major_vec(np.asarray(input), np.float16)
    lvw = np.concatenate(
        [
            _col_major_vec(np.asarray(llr)),
            _col_major_vec(np.asarray(llr_weight).reshape(E)),
        ],
        axis=1,
    )

    in_maps = []
    for c in range(N_CORES):
        rows = slice(c * R, (c + 1) * R)
        in_maps.append(
            {
                "wt": _prep_matrix(np.asarray(input_weight)[rows]),
                "mt": _prep_matrix(np.asarray(mask)[rows], f8),
                "et": _prep_matrix(np.asarray(llr_expander)[rows], f8),
                "xcm": xcm,
                "lvw": lvw,
            }
        )
    return in_maps


def kernel(input, input_weight, mask, llr, llr_weight, llr_expander):
    from concourse.bass_utils import run_bass_kernel_spmd

    nc = _build_program()
    in_maps = _make_in_maps(input, input_weight, mask, llr, llr_weight, llr_expander)
    res = run_bass_kernel_spmd(nc, in_maps, core_ids=list(range(N_CORES)))
    out = np.concatenate([res.results[c]["y"] for c in range(N_CORES)])
    return out.reshape(E, 1).astype(np.float32)


# revision 28
# speedup vs baseline: 1.0186x; 1.0186x over previous
"""Trainium2 Bass kernel for nn_BeliefPropagationCV (belief-propagation edge update).

Computes  y = 0.5 * ((mask * input_weight) @ input + llr_expander @ (llr_weight * llr))
for E = 4096 edges on 8 NeuronCores.

Sharding: row-shard the three [E, E] matrices (split output dim E into 8 slices of
512 rows); replicate the small vectors. Each core's shard is fed TRANSPOSED
(contraction dim j on SBUF partitions) so the TensorEngine performs the
x-weighted reduction directly via PSUM accumulation:

    y[i] = sum_j (mask.T*W.T)[j,i] * x[j] + sum_j E.T[j,i] * v[j],  v = llr_w*llr

Per 128-row j-chunk k: matmul(psum[1,512], lhsT=x[:,k:k+1], rhs=P_tile) accumulates.
The only elementwise work is one fp16 DVE multiply (mask ⊙ W) per tile.

mask / llr_expander are 0/1-valued, so the host-side fp16 cast is exact; W/x/v
are rounded to fp16 (~2^-11 relative), accumulation is fp32 in PSUM.
"""

import numpy as np

E = 4096
N_CORES = 8
R = E // N_CORES      # 512 output rows per core
P = 128               # SBUF partitions
K = E // P            # 32 contraction chunks of 128
# Ragged outer tiles (in 128-row contraction chunks): big tiles stream at
# line rate; the last tiles are small so the multiply+matmul chain hanging
# off the final DMA is short.
TILES = [4, 4, 4, 4, 4, 4, 4, 2, 2]
assert sum(TILES) == K
OFFS = [sum(TILES[:i]) for i in range(len(TILES))]  # first chunk of each tile


def _build_program():
    import concourse.bass as bass
    import concourse.tile as tile
    from concourse import bacc, mybir
    from contextlib import ExitStack

    f8 = mybir.dt.float8e4
    f16 = mybir.dt.float16
    f32 = mybir.dt.float32

    nc = bacc.Bacc(None)
    # Flat shard layouts: per outer tile a [P, cpo*R] contiguous block.
    wt = nc.dram_tensor("wt", [K * P * R], f16, kind="ExternalInput")
    # mask / llr_expander are 0/1-valued: fp8_e4m3 is exact and halves traffic.
    mt = nc.dram_tensor("mt", [K * P * R], f8, kind="ExternalInput")
    et = nc.dram_tensor("et", [K * P * R], f8, kind="ExternalInput")
    xcm = nc.dram_tensor("xcm", [P, K], f16, kind="ExternalInput")
    lvw = nc.dram_tensor("lvw", [P, 2 * K], f32, kind="ExternalInput")
    y = nc.dram_tensor("y", [R], f32, kind="ExternalOutput")

    def tile_ap(dram, g):
        off = OFFS[g] * P * R
        n = TILES[g] * P * R
        return dram[off : off + n].rearrange("(p f) -> p f", p=P)

    with ExitStack() as ctx:
        tc = ctx.enter_context(tile.TileContext(nc))
        # bufs = all tiles resident at once (about 14 MB of SBUF) so the DMA
        # stream never stalls on slot reuse.
        NT = len(TILES)
        singles = ctx.enter_context(tc.tile_pool(name="singles", bufs=1))
        wp = ctx.enter_context(tc.tile_pool(name="wp", bufs=NT))
        mp = ctx.enter_context(tc.tile_pool(name="mp", bufs=NT))
        ep = ctx.enter_context(tc.tile_pool(name="ep", bufs=NT))
        pp = ctx.enter_context(tc.tile_pool(name="pp", bufs=NT))
        psp = ctx.enter_context(tc.tile_pool(name="psp", bufs=1, space="PSUM"))

        # PE warm-up: the HAM clock gate keeps the PE at 1.2 GHz until it has
        # been busy ~3.4us. Run zero matmuls into a scratch PSUM bank during
        # the DMA ramp so the real matmuls run at 2.4 GHz.
        zmov = singles.tile([P, R], f16)
        nc.vector.memset(zmov, 0.0)
        zps = psp.tile([1, R], f32)
        for _ in range(28):
            nc.tensor.matmul(zps, zmov[:, :1], zmov, start=True, stop=True)

        # Small replicated vectors first on the ACT ring (tiny). Column-major
        # ([p, k] = elem k*128+p) so contraction chunk k is SBUF column k.
        xh = singles.tile([P, K], f16)
        nc.scalar.dma_start(out=xh, in_=xcm[:, :])
        lvf = singles.tile([P, 2 * K], f32)
        nc.scalar.dma_start(out=lvf, in_=lvw[:, :])
        vh = singles.tile([P, K], f16)
        nc.vector.tensor_mul(vh, lvf[:, :K], lvf[:, K:])

        # Per-tile interleaved loads: W on the SP ring; mask+expander on the
        # ACT ring right behind the small vectors.
        w_sbs, m_sbs, e_sbs = [], [], []
        for g in range(NT):
            fr = TILES[g] * R
            w_sb = wp.tile([P, fr], f16, tag="w_sb")
            nc.sync.dma_start(out=w_sb, in_=tile_ap(wt, g))
            m_sb = mp.tile([P, fr], f8, tag="m_sb")
            nc.scalar.dma_start(out=m_sb, in_=tile_ap(mt, g))
            e_sb = ep.tile([P, fr], f8, tag="e_sb")
            nc.scalar.dma_start(out=e_sb, in_=tile_ap(et, g))
            w_sbs.append(w_sb); m_sbs.append(m_sb); e_sbs.append(e_sb)

        ps = psp.tile([1, R], f32)
        n_mm = K * 2
        i_mm = 0
        for g in range(NT):
            cpo = TILES[g]
            # Mixed-dtype multiply fp16 W x fp8 mask -> fp16. The DVE runs at
            # 1 elem/cycle/lane on mixed dtypes, so for big tiles GpSimd takes
            # the last chunk in parallel; the first chunk is its own multiply
            # so its matmuls overlap the rest.
            p_sb = pp.tile([P, cpo * R], f16, tag="p_sb")
            if cpo >= 3:
                gsl = bass.ts(cpo - 1, R)
                nc.gpsimd.tensor_mul(p_sb[:, gsl], w_sbs[g][:, gsl], m_sbs[g][:, gsl])
                pieces = ((0, 1), (1, cpo - 1))
            else:
                pieces = tuple((c, c + 1) for c in range(cpo))
            for lo, hi in pieces:
                hsl = bass.ds(lo * R, (hi - lo) * R)
                nc.vector.tensor_mul(p_sb[:, hsl], w_sbs[g][:, hsl], m_sbs[g][:, hsl])
            for c in range(cpo):
                k = OFFS[g] + c
                sl = bass.ts(c, R)
                nc.tensor.matmul(
                    ps, xh[:, k : k + 1], p_sb[:, sl],
                    start=(i_mm == 0), stop=(i_mm == n_mm - 1),
                )
                i_mm += 1
                nc.tensor.matmul(
                    ps, vh[:, k : k + 1], e_sbs[g][:, sl],
                    start=False, stop=(i_mm == n_mm - 1),
                )
                i_mm += 1

        # 0.5 * (term1 + term2) applied once on the tiny epilogue copy (DVE,
        # not ACT: using the scalar engine would pull in its activation-table
        # preamble load, delaying the ACT HWDGE ring's first data transfer).
        ysb = singles.tile([1, R], f32)
        nc.vector.tensor_scalar_mul(ysb, ps, 0.5)
        nc.sync.dma_start(out=y[:], in_=ysb)

    # bacc passes: splits multi-waits into event semaphores (TRN2 allows at
    # most one sync wait per instruction), register allocation, etc.
    nc.compile()
    return nc


def _prep_matrix(a_rows: np.ndarray, dtype=np.float16) -> np.ndarray:
    """[R, E] float -> flat [K*P*R]: per outer tile a [P, cpo*R] block with
    the contraction dim on partitions.

    block_g[p, c*R + i] = a_rows[i, (OFFS[g] + c)*P + p]
    """
    at = a_rows.astype(dtype).T.reshape(K, P, R)  # [k, p, i]
    blocks = []
    for g, cpo in enumerate(TILES):
        blk = at[OFFS[g] : OFFS[g] + cpo]         # [cpo, P, R]
        blocks.append(np.ascontiguousarray(blk.transpose(1, 0, 2)).reshape(-1))
    return np.concatenate(blocks)


def _f8_dtype():
    from concourse import mybir

    return mybir.dt.np(mybir.dt.float8e4)


def _col_major_vec(v: np.ndarray, dtype=np.float32) -> np.ndarray:
    """[E] -> [P, K] with [p, k] = v[k*P + p]."""
    return np.ascontiguousarray(v.reshape(K, P).T.astype(dtype))


def _make_in_maps(input, input_weight, mask, llr, llr_weight, llr_expander):
    f8 = _f8_dtype()
    xcm = _col_major_vec(np.asarray(input), np.float16)
    lvw = np.concatenate(
        [
            _col_major_vec(np.asarray(llr)),
            _col_major_vec(np.asarray(llr_weight).reshape(E)),
        ],
        axis=1,
    )

    in_maps = []
    for c in range(N_CORES):
        rows = slice(c * R, (c + 1) * R)
        in_maps.append(
            {
                "wt": _prep_matrix(np.asarray(input_weight)[rows]),
                "mt": _prep_matrix(np.asarray(mask)[rows], f8),
                "et": _prep_matrix(np.asarray(llr_expander)[rows], f8),
                "xcm": xcm,
                "lvw": lvw,
            }
        )
    return in_maps


def kernel(input, input_weight, mask, llr, llr_weight, llr_expander):
    from concourse.bass_utils import run_bass_kernel_spmd

    nc = _build_program()
    in_maps = _make_in_maps(input, input_weight, mask, llr, llr_weight, llr_expander)
    res = run_bass_kernel_spmd(nc, in_maps, core_ids=list(range(N_CORES)))
    out = np.concatenate([res.results[c]["y"] for c in range(N_CORES)])
    return out.reshape(E, 1).astype(np.float32)


# revision 29
# speedup vs baseline: 1.0291x; 1.0103x over previous
"""Trainium2 Bass kernel for nn_BeliefPropagationCV (belief-propagation edge update).

Computes  y = 0.5 * ((mask * input_weight) @ input + llr_expander @ (llr_weight * llr))
for E = 4096 edges on 8 NeuronCores.

Sharding: row-shard the three [E, E] matrices (split output dim E into 8 slices of
512 rows); replicate the small vectors. Each core's shard is fed TRANSPOSED
(contraction dim j on SBUF partitions) so the TensorEngine performs the
x-weighted reduction directly via PSUM accumulation:

    y[i] = sum_j (mask.T*W.T)[j,i] * x[j] + sum_j E.T[j,i] * v[j],  v = llr_w*llr

Per 128-row j-chunk k: matmul(psum[1,512], lhsT=x[:,k:k+1], rhs=P_tile) accumulates.
The only elementwise work is one mixed-dtype multiply (mask ⊙ W) per tile,
split across the DVE and GpSimd engines.

mask / llr_expander are 0/1-valued, so the host-side fp8_e4m3 cast is exact
(and halves their HBM traffic); W/x/v are rounded to fp16 (~2^-11 relative),
accumulation is fp32 in PSUM. Per-core HBM traffic is 8.4 MB; measured ~48 us
on HW against a ~24 us pure-DMA roofline plus ~21 us fixed NEFF overhead
(preamble barrier + semaphore-clear postamble, measured on a trivial kernel).
"""

import numpy as np

E = 4096
N_CORES = 8
R = E // N_CORES      # 512 output rows per core
P = 128               # SBUF partitions
K = E // P            # 32 contraction chunks of 128
# Ragged outer tiles (in 128-row contraction chunks): big tiles stream at
# line rate; the last tiles are small so the multiply+matmul chain hanging
# off the final DMA is short.
TILES = [4, 4, 4, 4, 4, 4, 4, 2, 2]
assert sum(TILES) == K
OFFS = [sum(TILES[:i]) for i in range(len(TILES))]  # first chunk of each tile


def _build_program():
    import concourse.bass as bass
    import concourse.tile as tile
    from concourse import bacc, mybir
    from contextlib import ExitStack

    f8 = mybir.dt.float8e4
    f16 = mybir.dt.float16
    f32 = mybir.dt.float32

    nc = bacc.Bacc(None)
    # Flat shard layouts: per outer tile a [P, cpo*R] contiguous block.
    wt = nc.dram_tensor("wt", [K * P * R], f16, kind="ExternalInput")
    # mask / llr_expander are 0/1-valued: fp8_e4m3 is exact and halves traffic.
    mt = nc.dram_tensor("mt", [K * P * R], f8, kind="ExternalInput")
    et = nc.dram_tensor("et", [K * P * R], f8, kind="ExternalInput")
    xcm = nc.dram_tensor("xcm", [P, K], f16, kind="ExternalInput")
    lvw = nc.dram_tensor("lvw", [P, 2 * K], f32, kind="ExternalInput")
    y = nc.dram_tensor("y", [R], f32, kind="ExternalOutput")

    def tile_ap(dram, g):
        off = OFFS[g] * P * R
        n = TILES[g] * P * R
        return dram[off : off + n].rearrange("(p f) -> p f", p=P)

    with ExitStack() as ctx:
        tc = ctx.enter_context(tile.TileContext(nc))
        # bufs = all tiles resident at once (about 14 MB of SBUF) so the DMA
        # stream never stalls on slot reuse.
        NT = len(TILES)
        singles = ctx.enter_context(tc.tile_pool(name="singles", bufs=1))
        wp = ctx.enter_context(tc.tile_pool(name="wp", bufs=NT))
        mp = ctx.enter_context(tc.tile_pool(name="mp", bufs=NT))
        ep = ctx.enter_context(tc.tile_pool(name="ep", bufs=NT))
        pp = ctx.enter_context(tc.tile_pool(name="pp", bufs=NT))
        psp = ctx.enter_context(tc.tile_pool(name="psp", bufs=1, space="PSUM"))

        # PE warm-up: the HAM clock gate keeps the PE at 1.2 GHz until it has
        # been busy ~3.4us. Run zero matmuls into a scratch PSUM bank during
        # the DMA ramp so the real matmuls run at 2.4 GHz.
        zmov = singles.tile([P, R], f16)
        nc.vector.memset(zmov, 0.0)
        zps = psp.tile([1, R], f32)
        for _ in range(28):
            nc.tensor.matmul(zps, zmov[:, :1], zmov, start=True, stop=True)

        # Small replicated vectors first on the ACT ring (tiny). Column-major
        # ([p, k] = elem k*128+p) so contraction chunk k is SBUF column k.
        xh = singles.tile([P, K], f16)
        nc.scalar.dma_start(out=xh, in_=xcm[:, :])
        lvf = singles.tile([P, 2 * K], f32)
        nc.scalar.dma_start(out=lvf, in_=lvw[:, :])
        vh = singles.tile([P, K], f16)
        nc.vector.tensor_mul(vh, lvf[:, :K], lvf[:, K:])

        # Per-tile interleaved loads: W on the SP ring; mask+expander on the
        # ACT ring right behind the small vectors.
        w_sbs, m_sbs, e_sbs = [], [], []
        for g in range(NT):
            fr = TILES[g] * R
            w_sb = wp.tile([P, fr], f16, tag="w_sb")
            nc.sync.dma_start(out=w_sb, in_=tile_ap(wt, g))
            m_sb = mp.tile([P, fr], f8, tag="m_sb")
            nc.scalar.dma_start(out=m_sb, in_=tile_ap(mt, g))
            e_sb = ep.tile([P, fr], f8, tag="e_sb")
            nc.scalar.dma_start(out=e_sb, in_=tile_ap(et, g))
            w_sbs.append(w_sb); m_sbs.append(m_sb); e_sbs.append(e_sb)

        ps = psp.tile([1, R], f32)
        n_mm = K * 2
        i_mm = 0
        for g in range(NT):
            cpo = TILES[g]
            # Mixed-dtype multiply fp16 W x fp8 mask -> fp16. The DVE runs at
            # 1 elem/cycle/lane on mixed dtypes, so for big tiles GpSimd takes
            # the last chunk in parallel; the first chunk is its own multiply
            # so its matmuls overlap the rest.
            p_sb = pp.tile([P, cpo * R], f16, tag="p_sb")
            if cpo >= 3:
                gsl = bass.ts(cpo - 1, R)
                nc.gpsimd.tensor_mul(p_sb[:, gsl], w_sbs[g][:, gsl], m_sbs[g][:, gsl])
                pieces = ((0, 1), (1, cpo - 1))
            else:
                pieces = tuple((c, c + 1) for c in range(cpo))
            for lo, hi in pieces:
                hsl = bass.ds(lo * R, (hi - lo) * R)
                nc.vector.tensor_mul(p_sb[:, hsl], w_sbs[g][:, hsl], m_sbs[g][:, hsl])
            for c in range(cpo):
                k = OFFS[g] + c
                sl = bass.ts(c, R)
                nc.tensor.matmul(
                    ps, xh[:, k : k + 1], p_sb[:, sl],
                    start=(i_mm == 0), stop=(i_mm == n_mm - 1),
                )
                i_mm += 1
                nc.tensor.matmul(
                    ps, vh[:, k : k + 1], e_sbs[g][:, sl],
                    start=False, stop=(i_mm == n_mm - 1),
                )
                i_mm += 1

        # 0.5 * (term1 + term2) applied once on the tiny epilogue copy (DVE,
        # not ACT: using the scalar engine would pull in its activation-table
        # preamble load, delaying the ACT HWDGE ring's first data transfer).
        ysb = singles.tile([1, R], f32)
        nc.vector.tensor_scalar_mul(ysb, ps, 0.5)
        nc.sync.dma_start(out=y[:], in_=ysb)

    # bacc passes: splits multi-waits into event semaphores (TRN2 allows at
    # most one sync wait per instruction), register allocation, etc.
    nc.compile()
    return nc


def _prep_matrix(a_rows: np.ndarray, dtype=np.float16) -> np.ndarray:
    """[R, E] float -> flat [K*P*R]: per outer tile a [P, cpo*R] block with
    the contraction dim on partitions.

    block_g[p, c*R + i] = a_rows[i, (OFFS[g] + c)*P + p]
    """
    at = a_rows.astype(dtype).T.reshape(K, P, R)  # [k, p, i]
    blocks = []
    for g, cpo in enumerate(TILES):
        blk = at[OFFS[g] : OFFS[g] + cpo]         # [cpo, P, R]
        blocks.append(np.ascontiguousarray(blk.transpose(1, 0, 2)).reshape(-1))
    return np.concatenate(blocks)


def _f8_dtype():
    from concourse import mybir

    return mybir.dt.np(mybir.dt.float8e4)


def _col_major_vec(v: np.ndarray, dtype=np.float32) -> np.ndarray:
    """[E] -> [P, K] with [p, k] = v[k*P + p]."""
    return np.ascontiguousarray(v.reshape(K, P).T.astype(dtype))


def _make_in_maps(input, input_weight, mask, llr, llr_weight, llr_expander):
    f8 = _f8_dtype()
    xcm = _col_major_vec(np.asarray(input), np.float16)
    lvw = np.concatenate(
        [
            _col_major_vec(np.asarray(llr)),
            _col_major_vec(np.asarray(llr_weight).reshape(E)),
        ],
        axis=1,
    )

    in_maps = []
    for c in range(N_CORES):
        rows = slice(c * R, (c + 1) * R)
        in_maps.append(
            {
                "wt": _prep_matrix(np.asarray(input_weight)[rows]),
                "mt": _prep_matrix(np.asarray(mask)[rows], f8),
                "et": _prep_matrix(np.asarray(llr_expander)[rows], f8),
                "xcm": xcm,
                "lvw": lvw,
            }
        )
    return in_maps


def kernel(input, input_weight, mask, llr, llr_weight, llr_expander):
    from concourse.bass_utils import run_bass_kernel_spmd

    nc = _build_program()
    in_maps = _make_in_maps(input, input_weight, mask, llr, llr_weight, llr_expander)
    res = run_bass_kernel_spmd(nc, in_maps, core_ids=list(range(N_CORES)))
    out = np.concatenate([res.results[c]["y"] for c in range(N_CORES)])
    return out.reshape(E, 1).astype(np.float32)
